# revision 6
# baseline (speedup 1.0000x reference)
"""GCN layer kernel for Trainium2 (8 NeuronCores, SPMD).

out = relu((H + scatter_add(H[src], dst)) @ W)

Sharding: nodes (dst) partitioned across 8 cores (N padded 100000 -> 100352 =
784 blocks of 128; 98 blocks/core). Edges are bucketed by destination block
and laid out into fixed-capacity per-block slots (CAP edges/block, multiple of
128); message features H[src] are sharded per-destination (gathered during
input sharding, bf16) since this runtime exposes no working device-side
indexed-DMA path (custom GPSIMD ucode libraries unavailable; vector dynamic
DGE offsets broken).

Device per block b:
  psum[f, n]  = I128 @ HT_b          (f32 identity matmul = H self-term)
             += sum_t msgs_t^T @ onehot_t   (bf16, f32 accumulate)
  onehot_t[e, n] = (iota[n] == dst_local[e])  built on DVE
  out[n, :]   = relu((psum^T) @ W)   via ACT copy (bf16 cast) + PE + ACT relu
"""
import numpy as np
import ml_dtypes

import concourse.bacc as bacc
import concourse.mybir as mybir
from concourse.tile import TileContext
from concourse.bass_utils import run_bass_kernel_spmd

N = 100000
D_IN = 128
D_OUT = 256
N_CORES = 8
N_PAD = 100352
NODES_PER_CORE = N_PAD // N_CORES        # 12544
BLOCKS_PER_CORE = NODES_PER_CORE // 128  # 98
GB = 8                                   # dst blocks per msgs DMA group

bf16 = ml_dtypes.bfloat16


def _group_sizes():
    sizes = []
    b = BLOCKS_PER_CORE
    while b > 0:
        sizes.append(min(GB, b))
        b -= GB
    return sizes


def build_program(cap_tiles: int):
    T = cap_tiles                        # tiles (of 128 edge slots) per block
    total_tiles = BLOCKS_PER_CORE * T

    nc = bacc.Bacc("TRN2", target_bir_lowering=False)
    msgs_d = nc.declare_dram_parameter("msgs", [128, total_tiles, D_IN], mybir.dt.bfloat16, isOutput=False)
    ht = nc.declare_dram_parameter("ht", [BLOCKS_PER_CORE, 128, 128], mybir.dt.float32, isOutput=False)
    sdst = nc.declare_dram_parameter("sdst", [128, total_tiles], mybir.dt.float32, isOutput=False)
    wmat = nc.declare_dram_parameter("wmat", [D_IN, D_OUT], mybir.dt.bfloat16, isOutput=False)
    iota_d = nc.declare_dram_parameter("iota", [128, 128], mybir.dt.float32, isOutput=False)
    ident_d = nc.declare_dram_parameter("ident", [128, 128], mybir.dt.float32, isOutput=False)
    out = nc.declare_dram_parameter("out", [NODES_PER_CORE, D_OUT], mybir.dt.float32, isOutput=True)

    with TileContext(nc) as tc:
        with (
            tc.tile_pool(name="const", bufs=1) as constp,
            tc.tile_pool(name="sdstp", bufs=1) as sdstp,
            tc.tile_pool(name="msgs", bufs=2) as msgsp,
            tc.tile_pool(name="htp", bufs=3) as htp,
            tc.tile_pool(name="oh", bufs=4) as ohp,
            tc.tile_pool(name="xt", bufs=3) as xtp,
            tc.tile_pool(name="outp", bufs=3) as outp,
            tc.tile_pool(name="ps", bufs=3, space="PSUM") as psp,
            tc.tile_pool(name="ps2", bufs=2, space="PSUM") as ps2p,
        ):
            ident = constp.tile([128, 128], mybir.dt.float32)
            nc.sync.dma_start(out=ident[:, :], in_=ident_d[:, :])
            iota_b = constp.tile([128, 128], mybir.dt.float32)
            nc.sync.dma_start(out=iota_b[:, :], in_=iota_d[:, :])
            w_t = constp.tile([D_IN, D_OUT], mybir.dt.bfloat16)
            nc.sync.dma_start(out=w_t[:, :], in_=wmat[:, :])
            sdst_t = sdstp.tile([128, total_tiles], mybir.dt.float32)
            nc.sync.dma_start(out=sdst_t[:, :], in_=sdst[:, :])

            blk0 = 0
            for gsz in _group_sizes():
                g_tiles = gsz * T
                msgs_t = msgsp.tile([128, g_tiles, D_IN], mybir.dt.bfloat16, tag="msgs")
                nc.sync.dma_start(
                    out=msgs_t[:, :, :],
                    in_=msgs_d[:, blk0 * T : blk0 * T + g_tiles, :],
                )
                for b in range(gsz):
                    blk = blk0 + b
                    psum = psp.tile([128, 128], mybir.dt.float32, tag="ps")
                    ht_t = htp.tile([128, 128], mybir.dt.float32, tag="ht")
                    nc.sync.dma_start(out=ht_t[:, :], in_=ht[blk, :, :])
                    nc.tensor.matmul(out=psum[:, :], lhsT=ident[:, :], rhs=ht_t[:, :],
                                     start=True, stop=False, skip_group_check=True)
                    for t in range(T):
                        gcol = blk * T + t
                        oh_t = ohp.tile([128, 128], mybir.dt.bfloat16, tag="oh")
                        nc.vector.tensor_scalar(
                            out=oh_t[:, :], in0=iota_b[:, :],
                            scalar1=sdst_t[:, gcol : gcol + 1], scalar2=None,
                            op0=mybir.AluOpType.is_equal,
                        )
                        nc.tensor.matmul(
                            out=psum[:, :], lhsT=msgs_t[:, b * T + t, :], rhs=oh_t[:, :],
                            start=False, stop=(t == T - 1), skip_group_check=True,
                        )
                    xt_t = xtp.tile([128, 128], mybir.dt.bfloat16, tag="xt")
                    nc.scalar.activation(out=xt_t[:, :], in_=psum[:, :],
                                         func=mybir.ActivationFunctionType.Copy)
                    psum2 = ps2p.tile([128, D_OUT], mybir.dt.float32, tag="ps2")
                    nc.tensor.matmul(out=psum2[:, :], lhsT=xt_t[:, :], rhs=w_t[:, :],
                                     start=True, stop=True)
                    out_t = outp.tile([128, D_OUT], mybir.dt.float32, tag="out")
                    nc.scalar.activation(out=out_t[:, :], in_=psum2[:, :],
                                         func=mybir.ActivationFunctionType.Relu)
                    nc.sync.dma_start(
                        out=out[blk * 128 : (blk + 1) * 128, :], in_=out_t[:, :]
                    )
                blk0 += gsz
    nc.finalize()
    return nc


def preprocess(H, edge_index, W):
    src = np.asarray(edge_index[0], dtype=np.int64)
    dst = np.asarray(edge_index[1], dtype=np.int64)
    H = np.asarray(H, dtype=np.float32)
    W = np.asarray(W, dtype=np.float32)

    core = dst // NODES_PER_CORE
    block_g = dst // 128                  # global block id 0..783
    dst_local = dst % 128

    nseg = N_PAD // 128                   # 784 blocks globally
    counts = np.bincount(block_g, minlength=nseg)
    cap = int(np.ceil(max(counts.max(), 1) / 128) * 128)
    cap_tiles = cap // 128
    total_tiles = BLOCKS_PER_CORE * cap_tiles

    order = np.argsort(block_g, kind="stable")
    sorted_b = block_g[order]
    seg_starts = np.searchsorted(sorted_b, np.arange(nseg))
    rank = np.arange(len(src)) - seg_starts[sorted_b]

    H_pad = np.zeros((N_PAD, D_IN), dtype=np.float32)
    H_pad[:N] = H
    H_b = H_pad.astype(bf16)

    iota = np.tile(np.arange(128, dtype=np.float32), (128, 1))
    ident = np.eye(128, dtype=np.float32)
    wmat = W.astype(bf16)

    e_src = src[order]
    e_dstl = dst_local[order]
    e_blk = sorted_b
    # slot within core's slot space: (block_in_core * cap + rank); tile-major
    # layout: slot s -> tile s//128, partition s%128
    in_maps = []
    for c_id in range(N_CORES):
        lo = np.searchsorted(sorted_b, c_id * BLOCKS_PER_CORE)
        hi = np.searchsorted(sorted_b, (c_id + 1) * BLOCKS_PER_CORE)
        blk_in_core = e_blk[lo:hi] - c_id * BLOCKS_PER_CORE
        s = blk_in_core * cap + rank[lo:hi]
        msgs = np.zeros((BLOCKS_PER_CORE * cap, D_IN), dtype=bf16)
        msgs[s] = H_b[e_src[lo:hi]]
        # device layout: [128 partitions, total_tiles, D]; slot s -> tile
        # s//128, partition s%128
        msgs = np.ascontiguousarray(
            msgs.reshape(total_tiles, 128, D_IN).transpose(1, 0, 2)
        )
        sdst_flat = np.full(BLOCKS_PER_CORE * cap, -1.0, dtype=np.float32)
        sdst_flat[s] = e_dstl[lo:hi].astype(np.float32)
        sdst_arr = np.ascontiguousarray(sdst_flat.reshape(total_tiles, 128).T)
        hcore = H_pad[c_id * NODES_PER_CORE : (c_id + 1) * NODES_PER_CORE]
        ht_arr = np.ascontiguousarray(
            hcore.T.reshape(128, BLOCKS_PER_CORE, 128).transpose(1, 0, 2)
        )
        in_maps.append({
            "msgs": msgs,
            "ht": ht_arr,
            "sdst": sdst_arr,
            "wmat": wmat,
            "iota": iota,
            "ident": ident,
        })
    return in_maps, cap_tiles


_PROGRAM_CACHE = {}


def kernel(H, edge_index, W):
    in_maps, cap_tiles = preprocess(H, edge_index, W)
    nc = _PROGRAM_CACHE.get(cap_tiles)
    if nc is None:
        nc = build_program(cap_tiles)
        _PROGRAM_CACHE[cap_tiles] = nc
    res = run_bass_kernel_spmd(nc, in_maps, list(range(N_CORES)))
    out = np.concatenate([res.results[i]["out"] for i in range(N_CORES)], axis=0)
    return np.ascontiguousarray(out[:N])


# revision 7
# speedup vs baseline: 1.2011x; 1.2011x over previous
"""GCN layer kernel for Trainium2 (8 NeuronCores, SPMD).

out = relu((H + scatter_add(H[src], dst)) @ W)

Sharding: nodes (dst) partitioned across 8 cores (N padded 100000 -> 100352 =
784 blocks of 128; 98 blocks/core). Edges are bucketed by destination block
and laid out into fixed-capacity per-block slots (CAP edges/block, multiple of
128); message features H[src] are sharded per-destination (gathered during
input sharding, bf16) since this runtime exposes no working device-side
indexed-DMA path (custom GPSIMD ucode libraries unavailable; vector dynamic
DGE offsets broken).

Device per block b:
  psum[f, n]  = I128 @ HT_b          (f32 identity matmul = H self-term)
             += sum_t msgs_t^T @ onehot_t   (bf16, f32 accumulate)
  onehot_t[e, n] = (iota[n] == dst_local[e])  built on DVE
  out[n, :]   = relu((psum^T) @ W)   via ACT copy (bf16 cast) + PE + ACT relu
"""
import numpy as np
import ml_dtypes

import concourse.bacc as bacc
import concourse.mybir as mybir
from concourse.tile import TileContext
from concourse.bass_utils import run_bass_kernel_spmd

N = 100000
D_IN = 128
D_OUT = 256
N_CORES = 8
N_PAD = 100352
NODES_PER_CORE = N_PAD // N_CORES        # 12544
BLOCKS_PER_CORE = NODES_PER_CORE // 128  # 98
GB = 8                                   # dst blocks per msgs DMA group

bf16 = ml_dtypes.bfloat16


def _group_sizes():
    sizes = []
    b = BLOCKS_PER_CORE
    while b > 0:
        sizes.append(min(GB, b))
        b -= GB
    return sizes


def build_program(cap_tiles: int):
    T = cap_tiles                        # tiles (of 128 edge slots) per block
    total_tiles = BLOCKS_PER_CORE * T

    nc = bacc.Bacc("TRN2", target_bir_lowering=False)
    msgs_d = nc.declare_dram_parameter("msgs", [128, total_tiles, D_IN], mybir.dt.bfloat16, isOutput=False)
    ht = nc.declare_dram_parameter("ht", [BLOCKS_PER_CORE, 128, 128], mybir.dt.float32, isOutput=False)
    sdst = nc.declare_dram_parameter("sdst", [128, total_tiles], mybir.dt.float32, isOutput=False)
    wmat = nc.declare_dram_parameter("wmat", [D_IN, D_OUT], mybir.dt.bfloat16, isOutput=False)
    iota_d = nc.declare_dram_parameter("iota", [128, 128], mybir.dt.bfloat16, isOutput=False)
    ident_d = nc.declare_dram_parameter("ident", [128, 128], mybir.dt.float32, isOutput=False)
    out = nc.declare_dram_parameter("out", [NODES_PER_CORE, D_OUT], mybir.dt.float32, isOutput=True)

    with TileContext(nc) as tc:
        with (
            tc.tile_pool(name="const", bufs=1) as constp,
            tc.tile_pool(name="sdstp", bufs=1) as sdstp,
            tc.tile_pool(name="msgs", bufs=2) as msgsp,
            tc.tile_pool(name="htp", bufs=4) as htp,
            tc.tile_pool(name="oh", bufs=8) as ohp,
            tc.tile_pool(name="xt", bufs=3) as xtp,
            tc.tile_pool(name="outp", bufs=3) as outp,
            tc.tile_pool(name="ps", bufs=4, space="PSUM") as psp,
            tc.tile_pool(name="ps2", bufs=3, space="PSUM") as ps2p,
        ):
            ident = constp.tile([128, 128], mybir.dt.float32)
            nc.sync.dma_start(out=ident[:, :], in_=ident_d[:, :])
            iota_b = constp.tile([128, 128], mybir.dt.bfloat16)
            nc.sync.dma_start(out=iota_b[:, :], in_=iota_d[:, :])
            w_t = constp.tile([D_IN, D_OUT], mybir.dt.bfloat16)
            nc.sync.dma_start(out=w_t[:, :], in_=wmat[:, :])
            sdst_t = sdstp.tile([128, total_tiles], mybir.dt.float32)
            nc.sync.dma_start(out=sdst_t[:, :], in_=sdst[:, :])

            blk0 = 0
            for gsz in _group_sizes():
                g_tiles = gsz * T
                msgs_t = msgsp.tile([128, g_tiles, D_IN], mybir.dt.bfloat16, tag="msgs")
                nc.sync.dma_start(
                    out=msgs_t[:, :, :],
                    in_=msgs_d[:, blk0 * T : blk0 * T + g_tiles, :],
                )
                for b in range(gsz):
                    blk = blk0 + b
                    psum = psp.tile([128, 128], mybir.dt.float32, tag="ps")
                    ht_t = htp.tile([128, 128], mybir.dt.float32, tag="ht")
                    nc.sync.dma_start(out=ht_t[:, :], in_=ht[blk, :, :])
                    nc.tensor.matmul(out=psum[:, :], lhsT=ident[:, :], rhs=ht_t[:, :],
                                     start=True, stop=False, skip_group_check=True)
                    for t in range(T):
                        gcol = blk * T + t
                        oh_t = ohp.tile([128, 128], mybir.dt.bfloat16, tag="oh")
                        nc.vector.tensor_scalar(
                            out=oh_t[:, :], in0=iota_b[:, :],
                            scalar1=sdst_t[:, gcol : gcol + 1], scalar2=None,
                            op0=mybir.AluOpType.is_equal,
                        )
                        nc.tensor.matmul(
                            out=psum[:, :], lhsT=msgs_t[:, b * T + t, :], rhs=oh_t[:, :],
                            start=False, stop=(t == T - 1), skip_group_check=True,
                        )
                    xt_t = xtp.tile([128, 128], mybir.dt.bfloat16, tag="xt")
                    nc.scalar.activation(out=xt_t[:, :], in_=psum[:, :],
                                         func=mybir.ActivationFunctionType.Copy)
                    psum2 = ps2p.tile([128, D_OUT], mybir.dt.float32, tag="ps2")
                    nc.tensor.matmul(out=psum2[:, :], lhsT=xt_t[:, :], rhs=w_t[:, :],
                                     start=True, stop=True)
                    out_t = outp.tile([128, D_OUT], mybir.dt.float32, tag="out")
                    nc.scalar.activation(out=out_t[:, :], in_=psum2[:, :],
                                         func=mybir.ActivationFunctionType.Relu)
                    nc.sync.dma_start(
                        out=out[blk * 128 : (blk + 1) * 128, :], in_=out_t[:, :]
                    )
                blk0 += gsz
    nc.finalize()
    return nc


def preprocess(H, edge_index, W):
    src = np.asarray(edge_index[0], dtype=np.int64)
    dst = np.asarray(edge_index[1], dtype=np.int64)
    H = np.asarray(H, dtype=np.float32)
    W = np.asarray(W, dtype=np.float32)

    core = dst // NODES_PER_CORE
    block_g = dst // 128                  # global block id 0..783
    dst_local = dst % 128

    nseg = N_PAD // 128                   # 784 blocks globally
    counts = np.bincount(block_g, minlength=nseg)
    cap = int(np.ceil(max(counts.max(), 1) / 128) * 128)
    cap_tiles = cap // 128
    total_tiles = BLOCKS_PER_CORE * cap_tiles

    order = np.argsort(block_g, kind="stable")
    sorted_b = block_g[order]
    seg_starts = np.searchsorted(sorted_b, np.arange(nseg))
    rank = np.arange(len(src)) - seg_starts[sorted_b]

    H_pad = np.zeros((N_PAD, D_IN), dtype=np.float32)
    H_pad[:N] = H
    H_b = H_pad.astype(bf16)

    iota = np.tile(np.arange(128, dtype=np.float32).astype(bf16), (128, 1))
    ident = np.eye(128, dtype=np.float32)
    wmat = W.astype(bf16)

    e_src = src[order]
    e_dstl = dst_local[order]
    e_blk = sorted_b
    # slot within core's slot space: (block_in_core * cap + rank); tile-major
    # layout: slot s -> tile s//128, partition s%128
    in_maps = []
    for c_id in range(N_CORES):
        lo = np.searchsorted(sorted_b, c_id * BLOCKS_PER_CORE)
        hi = np.searchsorted(sorted_b, (c_id + 1) * BLOCKS_PER_CORE)
        blk_in_core = e_blk[lo:hi] - c_id * BLOCKS_PER_CORE
        s = blk_in_core * cap + rank[lo:hi]
        msgs = np.zeros((BLOCKS_PER_CORE * cap, D_IN), dtype=bf16)
        msgs[s] = H_b[e_src[lo:hi]]
        # device layout: [128 partitions, total_tiles, D]; slot s -> tile
        # s//128, partition s%128
        msgs = np.ascontiguousarray(
            msgs.reshape(total_tiles, 128, D_IN).transpose(1, 0, 2)
        )
        sdst_flat = np.full(BLOCKS_PER_CORE * cap, -1.0, dtype=np.float32)
        sdst_flat[s] = e_dstl[lo:hi].astype(np.float32)
        sdst_arr = np.ascontiguousarray(sdst_flat.reshape(total_tiles, 128).T)
        hcore = H_pad[c_id * NODES_PER_CORE : (c_id + 1) * NODES_PER_CORE]
        ht_arr = np.ascontiguousarray(
            hcore.T.reshape(128, BLOCKS_PER_CORE, 128).transpose(1, 0, 2)
        )
        in_maps.append({
            "msgs": msgs,
            "ht": ht_arr,
            "sdst": sdst_arr,
            "wmat": wmat,
            "iota": iota,
            "ident": ident,
        })
    return in_maps, cap_tiles


_PROGRAM_CACHE = {}


def kernel(H, edge_index, W):
    in_maps, cap_tiles = preprocess(H, edge_index, W)
    nc = _PROGRAM_CACHE.get(cap_tiles)
    if nc is None:
        nc = build_program(cap_tiles)
        _PROGRAM_CACHE[cap_tiles] = nc
    res = run_bass_kernel_spmd(nc, in_maps, list(range(N_CORES)))
    out = np.concatenate([res.results[i]["out"] for i in range(N_CORES)], axis=0)
    return np.ascontiguousarray(out[:N])


# revision 8
# speedup vs baseline: 1.4854x; 1.2367x over previous
"""GCN layer kernel for Trainium2 (8 NeuronCores, SPMD).

out = relu((H + scatter_add(H[src], dst)) @ W)

Sharding: nodes (dst) partitioned across 8 cores (N padded 100000 -> 100352 =
784 blocks of 128; 98 blocks/core). Edge messages H[src] are gathered into a
per-destination-block slot layout during input sharding (bf16); this runtime
exposes no working device-side indexed-DMA path (custom GPSIMD ucode
libraries unavailable; vector dynamic DGE offsets broken), so the gather is
part of the host-side shard step.

Scatter-add without per-tile mask generation: within each 128-node block,
nodes are ranked by in-degree (host-side permutation) and every rank r is
padded to a fleet-wide slot run L[r] (sum L = T*128). The per-tile scatter
matrix ("staircase": slot -> rank column) is then identical for every block
and core, so it is shipped once as a small input and the PE streams it as the
moving matmul operand -- no DVE one-hot builds at all. The host un-permutes
the 128 output rows of each block after download.

Device per block b:
  psum[f, n]  = sum_t msgs_(b,t)^T @ stair_t     (bf16 matmuls, f32 accum)
  xt[f, n]    = bf16(psum + HT_b)                (DVE tensor_tensor, H f32 exact)
  out[n, :]   = relu(xt^T @ W)                   (PE + ACT relu)
"""
import numpy as np
import ml_dtypes

import concourse.bacc as bacc
import concourse.mybir as mybir
from concourse.tile import TileContext
from concourse.bass_utils import run_bass_kernel_spmd

N = 100000
D_IN = 128
D_OUT = 256
N_CORES = 8
N_PAD = 100352
NODES_PER_CORE = N_PAD // N_CORES        # 12544
BLOCKS_PER_CORE = NODES_PER_CORE // 128  # 98
GB = 8                                   # dst blocks per msgs DMA group

bf16 = ml_dtypes.bfloat16


def _group_sizes():
    sizes = []
    b = BLOCKS_PER_CORE
    while b > 0:
        sizes.append(min(GB, b))
        b -= GB
    return sizes


def build_program(T: int):
    total_tiles = BLOCKS_PER_CORE * T

    nc = bacc.Bacc("TRN2", target_bir_lowering=False)
    msgs_d = nc.declare_dram_parameter("msgs", [128, total_tiles, D_IN], mybir.dt.bfloat16, isOutput=False)
    ht = nc.declare_dram_parameter("ht", [128, NODES_PER_CORE], mybir.dt.float32, isOutput=False)
    stair_d = nc.declare_dram_parameter("stair", [128, T, 128], mybir.dt.bfloat16, isOutput=False)
    wmat = nc.declare_dram_parameter("wmat", [D_IN, D_OUT], mybir.dt.bfloat16, isOutput=False)
    out = nc.declare_dram_parameter("out", [NODES_PER_CORE, D_OUT], mybir.dt.float32, isOutput=True)

    with TileContext(nc) as tc:
        with (
            tc.tile_pool(name="const", bufs=1) as constp,
            tc.tile_pool(name="msgs", bufs=2) as msgsp,
            tc.tile_pool(name="xt", bufs=4) as xtp,
            tc.tile_pool(name="outp", bufs=4) as outp,
            tc.tile_pool(name="ps", bufs=4, space="PSUM") as psp,
            tc.tile_pool(name="ps2", bufs=3, space="PSUM") as ps2p,
        ):
            stair_t = constp.tile([128, T, 128], mybir.dt.bfloat16)
            nc.sync.dma_start(out=stair_t[:, :, :], in_=stair_d[:, :, :])
            w_t = constp.tile([D_IN, D_OUT], mybir.dt.bfloat16)
            nc.sync.dma_start(out=w_t[:, :], in_=wmat[:, :])
            ht_t = constp.tile([128, NODES_PER_CORE], mybir.dt.float32)
            nc.sync.dma_start(out=ht_t[:, :], in_=ht[:, :])

            blk0 = 0
            for gsz in _group_sizes():
                g_tiles = gsz * T
                msgs_t = msgsp.tile([128, g_tiles, D_IN], mybir.dt.bfloat16, tag="msgs")
                nc.sync.dma_start(
                    out=msgs_t[:, :, :],
                    in_=msgs_d[:, blk0 * T : blk0 * T + g_tiles, :],
                )
                for b in range(gsz):
                    blk = blk0 + b
                    psum = psp.tile([128, 128], mybir.dt.float32, tag="ps")
                    for t in range(T):
                        nc.tensor.matmul(
                            out=psum[:, :], lhsT=msgs_t[:, b * T + t, :],
                            rhs=stair_t[:, t, :],
                            start=(t == 0), stop=(t == T - 1),
                        )
                    xt_t = xtp.tile([128, 128], mybir.dt.bfloat16, tag="xt")
                    nc.vector.tensor_tensor(
                        out=xt_t[:, :], in0=psum[:, :],
                        in1=ht_t[:, blk * 128 : (blk + 1) * 128],
                        op=mybir.AluOpType.add,
                    )
                    psum2 = ps2p.tile([128, D_OUT], mybir.dt.float32, tag="ps2")
                    nc.tensor.matmul(out=psum2[:, :], lhsT=xt_t[:, :], rhs=w_t[:, :],
                                     start=True, stop=True)
                    out_t = outp.tile([128, D_OUT], mybir.dt.float32, tag="out")
                    nc.scalar.activation(out=out_t[:, :], in_=psum2[:, :],
                                         func=mybir.ActivationFunctionType.Relu)
                    nc.sync.dma_start(
                        out=out[blk * 128 : (blk + 1) * 128, :], in_=out_t[:, :]
                    )
                blk0 += gsz
    nc.finalize()
    return nc


def preprocess(H, edge_index, W):
    src = np.asarray(edge_index[0], dtype=np.int64)
    dst = np.asarray(edge_index[1], dtype=np.int64)
    H = np.asarray(H, dtype=np.float32)
    W = np.asarray(W, dtype=np.float32)
    E = len(src)

    nblk = N_PAD // 128                                   # 784
    deg = np.bincount(dst, minlength=N_PAD)
    deg_b = deg.reshape(nblk, 128)

    # per-block degree ranking (desc, stable)
    rank_order = np.argsort(-deg_b, axis=1, kind="stable")  # [nblk, 128]: node-local id per rank
    node_rank = np.empty_like(rank_order)
    np.put_along_axis(node_rank, rank_order, np.broadcast_to(np.arange(128), (nblk, 128)), axis=1)

    ranked_deg = np.take_along_axis(deg_b, rank_order, axis=1)
    L = ranked_deg.max(axis=0).astype(np.int64)           # fleet-wide run length per rank
    T = int(np.ceil(max(L.sum(), 1) / 128))
    L[-1] += T * 128 - L.sum()                            # absorb padding in the last rank
    cum = np.concatenate([[0], np.cumsum(L)]).astype(np.int64)  # [129]

    # staircase constants: slot s=t*128+p -> rank column r where cum[r]<=s<cum[r+1]
    slot_rank = np.searchsorted(cum, np.arange(T * 128), side="right") - 1
    stair = np.zeros((T * 128, 128), dtype=bf16)
    stair[np.arange(T * 128), slot_rank] = 1.0
    stair = np.ascontiguousarray(
        stair.reshape(T, 128, 128).transpose(1, 0, 2)     # [p, t, n]
    )

    # per-edge slot: block, rank of dst within block, k-th edge of that dst
    order = np.argsort(dst, kind="stable")                # group edges by dst node
    sorted_dst = dst[order]
    starts = np.searchsorted(sorted_dst, np.arange(N_PAD))
    k_within = np.arange(E) - starts[sorted_dst]          # edge index within its dst
    blk_of_edge = sorted_dst // 128
    r_of_edge = node_rank[blk_of_edge, sorted_dst % 128]
    slot_in_block = cum[r_of_edge] + k_within
    slot_global = blk_of_edge * (T * 128) + slot_in_block

    H_pad = np.zeros((N_PAD, D_IN), dtype=np.float32)
    H_pad[:N] = H
    H_b = H_pad.astype(bf16)
    wmat = W.astype(bf16)

    slots_per_core = BLOCKS_PER_CORE * T * 128
    e_src = src[order]
    in_maps = []
    for c_id in range(N_CORES):
        lo = np.searchsorted(sorted_dst, c_id * NODES_PER_CORE)
        hi = np.searchsorted(sorted_dst, (c_id + 1) * NODES_PER_CORE)
        s = slot_global[lo:hi] - c_id * slots_per_core
        msgs = np.zeros((slots_per_core, D_IN), dtype=bf16)
        msgs[s] = H_b[e_src[lo:hi]]
        msgs = np.ascontiguousarray(
            msgs.reshape(BLOCKS_PER_CORE * T, 128, D_IN).transpose(1, 0, 2)
        )
        hcore = H_pad[c_id * NODES_PER_CORE : (c_id + 1) * NODES_PER_CORE]
        # ht in ranked order so psum columns line up: column n of block b is
        # the rank-n node; un-permuted after download
        perm = (rank_order[c_id * BLOCKS_PER_CORE : (c_id + 1) * BLOCKS_PER_CORE]
                + np.arange(BLOCKS_PER_CORE)[:, None] * 128).ravel()
        ht_arr = np.ascontiguousarray(hcore[perm].T)       # [128 f, 12544] ranked
        in_maps.append({
            "msgs": msgs,
            "ht": ht_arr,
            "stair": stair,
            "wmat": wmat,
        })
    return in_maps, T, rank_order


_PROGRAM_CACHE = {}


def kernel(H, edge_index, W):
    in_maps, T, rank_order = preprocess(H, edge_index, W)
    nc = _PROGRAM_CACHE.get(T)
    if nc is None:
        nc = build_program(T)
        _PROGRAM_CACHE[T] = nc
    res = run_bass_kernel_spmd(nc, in_maps, list(range(N_CORES)))
    out = np.concatenate([res.results[i]["out"] for i in range(N_CORES)], axis=0)
    # un-permute: device row (b, n) holds the rank-n node of block b
    perm = (rank_order + np.arange(N_PAD // 128)[:, None] * 128).ravel()
    out_full = np.empty_like(out)
    out_full[perm] = out
    return np.ascontiguousarray(out_full[:N])
